# revision 1
# baseline (speedup 1.0000x reference)
"""Trainium2 Bass kernel for the 2-layer heterogeneous GAT (drug/cell) model.

Strategy (8 NeuronCores, SPMD single program):
  - L1: all three relations (dd, dc, cc) partitioned by DST node block per
    core; edge-softmax segments stay local. Projections of the (replicated)
    input features are done on every core. No collectives in L1.
  - hd = o_dd exactly (drug semantic attention over M=1 is identity);
    layer-2 o_dd is dead code (final head reads cells only) and is skipped.
  - Softmax max-subtraction is dropped (values are O(1), exact in fp32):
    out[v] = (sum_e exp(e)·fs[src]) / (sum_e exp(e)).  Both segment sums are
    computed with one PE matmul per 128-edge chunk: lhsT = one-hot(dstloc),
    rhs = [w*fs | w], accumulated in PSUM over the chunks of each dst tile.
  - L2: dc edges partitioned by SRC drug block (gather tables stay local),
    giving per-core partial sums over all cells; AllToAll + local reduce
    delivers each core its own cell block.  cc edges partitioned by dst with
    a replicated table built from an AllGather of hc1^T.
  - Gathers use the batched int16 dma_gather (GPSIMD mlp library);
    drug tables (40960 rows) are split in two 20480-row halves per gather.
"""
import sys
sys.path.insert(0, '/opt/trn_rl_repo')
import os
import numpy as np

import concourse.bacc as bacc
import concourse.bass as bass
import concourse.tile as tile
from concourse import mybir, library_config

F32 = mybir.dt.float32
I16 = mybir.dt.int16
P = 128
H = 8
FD = 256          # feature dim
ROWW = 320        # gather-table row width (f32), 1280B, %256B
ERG = 256         # er/el group column offset inside table rows
AluOp = mybir.AluOpType
Act = mybir.ActivationFunctionType

FULL_CFG = dict(Nd=40000, Nc=10000, NDP=40960, NCP=10240,
                DBLK=5120, CBLK=1280, HALF=20480, n_cores=8)

N_CORES = 8


def legalize_waits(nc):
    """Split multi-wait instructions into wait-carrying NOP chains.

    The walrus build here accepts at most one sync-wait command per
    instruction (two for EventSemaphore); Tile attaches several.  Inserting
    NOPs immediately before the instruction on the same engine is
    semantically identical.
    """
    n_split = 0
    for fn in nc.m.functions:
        for bb in fn.blocks:
            insts = bb.instructions
            new = []
            changed = False
            for inst in insts:
                si = inst.sync_info
                waits = list(si.on_wait) if si is not None else []
                cap = 2 if isinstance(inst, mybir.InstEventSemaphore) else 1
                if len(waits) > cap:
                    keep = waits[-cap:]
                    for w in waits[:-cap]:
                        nop = mybir.InstNoOp(
                            name=nc.get_next_instruction_name(),
                            engine=inst.engine,
                            sync_info=mybir.SyncInfo(on_wait=[w], on_update=[]),
                            bass_nofuse=True,
                        )
                        new.append(nop)
                        n_split += 1
                    inst.sync_info = mybir.SyncInfo(
                        on_wait=keep, on_update=list(si.on_update))
                    changed = True
                new.append(inst)
            if changed:
                bb.instructions = new
    return n_split


# --------------------------------------------------------------------------
# host-side prep
# --------------------------------------------------------------------------

def _fold_weights(ip, cfg):
    """Fold attention vectors into projection matrices; build device weights."""
    def wel(W, a):  # W [256,256], a [H,D] -> [256,H]
        return (W.reshape(FD, H, 32) * a[None]).sum(-1)
    W = {}
    Wsrc, Wdst, al, ar = ip['Wsrc'], ip['Wdst_dc'], ip['attn_l'], ip['attn_r']
    # L1
    W['WDD'] = np.concatenate([Wsrc[0, 0], wel(Wsrc[0, 0], al[0, 0])], 1)   # [256,264]
    W['WerDD'] = wel(Wsrc[0, 0], ar[0, 0])                                   # [256,8]
    W['WDC'] = np.concatenate([Wsrc[0, 1], wel(Wsrc[0, 1], al[0, 1])], 1)
    W['WCC1'] = np.concatenate([Wsrc[0, 2], wel(Wsrc[0, 2], al[0, 2]),
                                wel(Wsrc[0, 2], ar[0, 2]),
                                wel(Wdst[0], ar[0, 1])], 1)                  # [256,280]
    # L2 (o_dd dead: only dc,cc)
    W['WDC2'] = np.concatenate([Wsrc[1, 1], wel(Wsrc[1, 1], al[1, 1])], 1)
    W['WCC2'] = np.concatenate([Wsrc[1, 2], wel(Wsrc[1, 2], al[1, 2]),
                                wel(Wsrc[1, 2], ar[1, 2]),
                                wel(Wdst[1], ar[1, 1])], 1)                  # [256,280]
    def padto(a, n):
        return np.concatenate([a, np.zeros((FD, n - a.shape[1]), a.dtype)], 1)
    out = {}
    out['WDD'] = padto(W['WDD'], ROWW).astype(np.float32).reshape(2, P, ROWW)
    out['WerDD'] = padto(W['WerDD'], 64).astype(np.float32).reshape(2, P, 64)
    out['WDC'] = padto(W['WDC'], ROWW).astype(np.float32).reshape(2, P, ROWW)
    out['WCC1'] = padto(W['WCC1'], ROWW).astype(np.float32).reshape(2, P, ROWW)
    out['WDC2'] = padto(W['WDC2'], ROWW).astype(np.float32).reshape(2, P, ROWW)
    out['WCC2'] = padto(W['WCC2'], ROWW).astype(np.float32).reshape(2, P, ROWW)
    out['bias5'] = np.stack([ip['gat_bias'][0, 0], ip['gat_bias'][0, 1],
                             ip['gat_bias'][0, 2], ip['gat_bias'][1, 1],
                             ip['gat_bias'][1, 2]]).astype(np.float32)       # [5,256]
    out['semW1'] = np.stack([ip['sem_W1'][0, 1].reshape(2, P, P),
                             ip['sem_W1'][1, 1].reshape(2, P, P)]).astype(np.float32)
    out['semb1'] = np.stack([ip['sem_b1'][0, 1].reshape(P, 1),
                             ip['sem_b1'][1, 1].reshape(P, 1)]).astype(np.float32)
    out['semW2'] = np.stack([ip['sem_W2'][0, 1].reshape(P, 1),
                             ip['sem_W2'][1, 1].reshape(P, 1)]).astype(np.float32)
    out['dnnW1'] = ip['dnn_W1'].reshape(2, P, 32).astype(np.float32)
    out['dnnb1'] = ip['dnn_b1'].reshape(32, 1).astype(np.float32)
    out['dnnW2'] = ip['dnn_W2'].astype(np.float32)                           # [32,16]
    out['dnnb2'] = ip['dnn_b2'].reshape(16, 1).astype(np.float32)
    out['dnnW3'] = ip['dnn_W3'].astype(np.float32)                           # [16,1]
    out['dnnb3'] = ip['dnn_b3'].reshape(1, 1).astype(np.float32)
    return out


def _wrap16(vals):
    """int16 values (len%128==0) -> wrapped [128, len/16] layout."""
    n = len(vals)
    out = np.zeros((P, n // 16), np.int16)
    a = np.asarray(vals, np.int16).reshape(-1, 16).T      # [16, n/16]
    for g in range(8):
        out[g * 16:(g + 1) * 16, :] = a
    return out


def _prep_rel(src, dst, cfg, *, part, n_tiles, halves, src_off=None,
              er_local_blk=None):
    """Build the per-core edge schedule for one relation instance.

    part: ('dst', blk) or ('src', blk) — which endpoint picks the core.
          For 'dst', tiles are local (dst - c*blk); for 'src', tiles are
          global over the dst space.
    halves: half size for the src gather table (None = single half).
    src_off: per-core offset subtracted from src (for core-local tables).
    er_local_blk: if set, er idx = dst - c*blk (core-local er table).
    Returns dict with nch (per tile per half, equalized over cores) and
    per-core streams (srcidx wrapped, eridx wrapped, dstloc f32).
    """
    nco = cfg['n_cores']
    kind, blk = part
    per_core = []
    for c in range(nco):
        if kind == 'dst':
            m = (dst >= c * blk) & (dst < (c + 1) * blk)
            dl = dst[m] - c * blk
        else:
            m = (src >= c * blk) & (src < (c + 1) * blk)
            dl = dst[m]
        s = src[m] - (c * src_off if src_off else 0)
        er = dst[m] - (c * er_local_blk if er_local_blk else 0)
        tile_id = dl // P
        per_core.append((s, dl % P, tile_id, er))
    nch = np.zeros((n_tiles, 2 if halves else 1), np.int64)
    buckets = []
    for c in range(nco):
        s, dloc, tid, erv = per_core[c]
        bk = {}
        for t in range(n_tiles):
            mt = tid == t
            st, dt_, ee = s[mt], dloc[mt], erv[mt]
            if halves:
                m0 = st < halves
                groups = [(st[m0], dt_[m0], ee[m0]),
                          (st[~m0] - halves, dt_[~m0], ee[~m0])]
            else:
                groups = [(st, dt_, ee)]
            bk[t] = groups
            for h, (gs, gd, gg) in enumerate(groups):
                nch[t, h] = max(nch[t, h], (len(gs) + P - 1) // P)
        buckets.append(bk)
    nch = np.maximum(nch, 1)  # at least one chunk so the psum group exists
    tot = int(nch.sum())
    srcs, ers, dls = [], [], []
    for c in range(nco):
        bk = buckets[c]
        sw = np.zeros((P, tot * 8), np.int16)
        ew = np.zeros((P, tot * 8), np.int16)
        dv = np.full((P, tot), -1.0, np.float32)
        col = 0
        for t in range(n_tiles):
            tile_er, tile_dl = [], []
            for h, (gs, gd, gg) in enumerate(bk[t]):
                n = int(nch[t, h]) * P
                a = np.zeros(n, np.int64)
                a[:len(gs)] = gs
                assert a.max(initial=0) < 32768
                sw[:, col * 8:(col + int(nch[t, h])) * 8] = _wrap16(a)
                tile_er.append((gg, n))
                dpad = np.full(n, -1.0, np.float32)
                dpad[:len(gd)] = gd
                tile_dl.append(dpad)
                col += int(nch[t, h])
            ern = np.zeros(sum(x[1] for x in tile_er), np.int64)
            off = 0
            for gg, n in tile_er:
                ern[off:off + len(gg)] = gg
                off += n
            assert ern.max(initial=0) < 32768
            tcol0 = col - int(nch[t].sum())
            ew[:, tcol0 * 8:col * 8] = _wrap16(ern)
            dall = np.concatenate(tile_dl)
            dv[:, tcol0:col] = dall.reshape(-1, P).T
        srcs.append(sw); ers.append(ew); dls.append(dv)
    return dict(nch=nch, src=srcs, er=ers, dl=dls, tot=tot)


def host_prep(ip, cfg):
    W = _fold_weights(ip, cfg)
    nco = cfg['n_cores']
    DBLK, CBLK, HALF = cfg['DBLK'], cfg['CBLK'], cfg['HALF']
    NDP, NCP = cfg['NDP'], cfg['NCP']
    DD_T, CT, DC2_T = DBLK // P, CBLK // P, NCP // P

    dd = _prep_rel(ip['src_dd'], ip['dst_dd'], cfg, part=('dst', DBLK),
                   n_tiles=DD_T, halves=HALF, er_local_blk=DBLK)
    dc1 = _prep_rel(ip['src_dc'], ip['dst_dc'], cfg, part=('dst', CBLK),
                    n_tiles=CT, halves=HALF)
    cc1 = _prep_rel(ip['src_cc'], ip['dst_cc'], cfg, part=('dst', CBLK),
                    n_tiles=CT, halves=None)
    dc2 = _prep_rel(ip['src_dc'], ip['dst_dc'], cfg, part=('src', DBLK),
                    n_tiles=DC2_T, halves=None, src_off=DBLK)
    cc2 = _prep_rel(ip['src_cc'], ip['dst_cc'], cfg, part=('dst', CBLK),
                    n_tiles=CT, halves=None)

    featD = np.zeros((NDP, FD), np.float32); featD[:cfg['Nd']] = ip['feat_drug']
    featC = np.zeros((NCP, FD), np.float32); featC[:cfg['Nc']] = ip['feat_cell']
    featDT = np.ascontiguousarray(featD.T).reshape(2, P, NDP)
    featCT = np.ascontiguousarray(featC.T).reshape(2, P, NCP)

    iota = np.broadcast_to(np.arange(P, dtype=np.float32), (P, P)).copy()
    ident = np.eye(P, dtype=np.float32)

    base = dict(featDT=featDT, featCT=featCT, iota=iota, ident=ident, **W)
    in_maps = []
    for c in range(nco):
        m = dict(base)
        m['featDTloc'] = np.ascontiguousarray(
            featD[c * DBLK:(c + 1) * DBLK].T).reshape(2, P, DBLK)
        m['dd_src'] = dd['src'][c]; m['dd_er'] = dd['er'][c]; m['dd_dl'] = dd['dl'][c]
        m['dc1_src'] = dc1['src'][c]; m['dc1_er'] = dc1['er'][c]; m['dc1_dl'] = dc1['dl'][c]
        m['cc1_src'] = cc1['src'][c]; m['cc1_er'] = cc1['er'][c]; m['cc1_dl'] = cc1['dl'][c]
        m['dc2_src'] = dc2['src'][c]; m['dc2_er'] = dc2['er'][c]; m['dc2_dl'] = dc2['dl'][c]
        m['cc2_src'] = cc2['src'][c]; m['cc2_er'] = cc2['er'][c]; m['cc2_dl'] = cc2['dl'][c]
        in_maps.append(m)
    sched = dict(dd=dd['nch'], dc1=dc1['nch'], cc1=cc1['nch'],
                 dc2=dc2['nch'], cc2=cc2['nch'])
    return sched, in_maps


# --------------------------------------------------------------------------
# device program
# --------------------------------------------------------------------------

def _proj_pass(nc, sb, ps, lhsT_dram, n_tiles, jobs, wtiles):
    """Project features into gather tables.

    lhsT_dram: [2, P, NCOLS] transposed features (DRAM input AP).
    jobs: list of (table_dram, ncols_payload, rhs_key) writing
          table rows [tile*P:(tile+1)*P, :ncols].
    wtiles: dict rhs_key -> (sbuf tile [P, ncols] chunk0, chunk1)
    """
    for t in range(n_tiles):
        lh0 = sb.tile([P, P], F32, tag="projlh")
        lh1 = sb.tile([P, P], F32, tag="projlh")
        nc.sync.dma_start(lh0[:], lhsT_dram[0, :, t * P:(t + 1) * P])
        nc.sync.dma_start(lh1[:], lhsT_dram[1, :, t * P:(t + 1) * P])
        for tab, ncols, wkey in jobs:
            w0, w1 = wtiles[wkey]
            pp = ps.tile([P, ncols], F32, space="PSUM", tag="projps")
            nc.tensor.matmul(pp[:], lhsT=lh0[:], rhs=w0[:], start=True, stop=False)
            nc.tensor.matmul(pp[:], lhsT=lh1[:], rhs=w1[:], start=False, stop=True)
            ot = sb.tile([P, ncols], F32, tag="projout")
            nc.scalar.copy(ot[:], pp[:])
            nc.sync.dma_start(tab[t * P:(t + 1) * P, :ncols], ot[:])


def _edge_phase(nc, sb, ps, consts, tab_views, er_view, streams, nch,
                epilogue, ncols_er_rel, er_step=ROWW):
    """One relation's edge phase.

    tab_views: list of per-half table APs (rows x ROWW) for the src gather.
    er_view:  AP [rows, 64] whose rel col range [ncols_er_rel:+8] holds er.
    streams:  (src16, er16, dl) DRAM APs.
    nch:      [T, n_halves] chunk counts.
    epilogue: fn(t, psum_ap, sb) consuming the accumulated [P,264] psum.
    """
    src16, er16, dlf = streams
    iota = consts['iota']
    T = nch.shape[0]
    col = 0
    for t in range(T):
        tile_nch = int(nch[t].sum())
        G = sb.tile([P, tile_nch, ROWW], F32, tag="G")
        E = sb.tile([P, tile_nch, 64], F32, tag="E")
        DL = sb.tile([P, tile_nch], F32, tag="DL")
        SI = sb.tile([P, tile_nch * 8], I16, tag="SI")
        EI = sb.tile([P, tile_nch * 8], I16, tag="EI")
        nc.sync.dma_start(DL[:], dlf[:, col:col + tile_nch])
        nc.sync.dma_start(SI[:], src16[:, col * 8:(col + tile_nch) * 8])
        nc.sync.dma_start(EI[:], er16[:, col * 8:(col + tile_nch) * 8])
        # src gathers (per half), split to <=256 idxs per call (SWDGE ring)
        GCH = 2
        ccol = 0
        for h in range(nch.shape[1]):
            nh = int(nch[t, h])
            for b0 in range(0, nh, GCH):
                nb = min(GCH, nh - b0)
                nidx = nb * P
                c0 = ccol + b0
                nc.gpsimd.dma_gather(
                    G[:, c0:c0 + nb, :], tab_views[h],
                    SI[:, c0 * 8:(c0 + nb) * 8], nidx, nidx, ROWW)
            ccol += nh
        # er gather, same split
        for b0 in range(0, tile_nch, GCH):
            nb = min(GCH, tile_nch - b0)
            nidx = nb * P
            nc.gpsimd.dma_gather(
                E[:, b0:b0 + nb, :], er_view, EI[:, b0 * 8:(b0 + nb) * 8],
                nidx, nidx, 64, elem_step=er_step)
        # edge math, batched over the tile's chunks
        ww = sb.tile([P, tile_nch, 8], F32, tag="ww")
        nc.vector.tensor_tensor(
            out=ww[:], in0=G[:, :, ERG:ERG + 8],
            in1=E[:, :, ncols_er_rel:ncols_er_rel + 8], op=AluOp.add)
        nc.vector.scalar_tensor_tensor(
            out=ww[:], in0=ww[:], scalar=0.2, in1=ww[:],
            op0=AluOp.mult, op1=AluOp.max)
        nc.scalar.activation(ww[:], ww[:], Act.Exp)
        pp = ps.tile([P, 264], F32, space="PSUM", tag="edgeps")
        for k in range(tile_nch):
            S = sb.tile([P, P], F32, tag="S")
            nc.vector.tensor_scalar(S[:], iota[:], DL[:, k:k + 1], None,
                                    op0=AluOp.is_equal)
            rhs = sb.tile([P, 264], F32, tag="rhs")
            nc.vector.tensor_tensor(
                out=rhs[:, :FD].rearrange("p (h d) -> p h d", h=H),
                in0=G[:, k, :FD].rearrange("p (h d) -> p h d", h=H),
                in1=ww[:, k, :, None].to_broadcast([P, H, 32]),
                op=AluOp.mult)
            nc.vector.tensor_copy(rhs[:, FD:FD + 8], ww[:, k, :])
            nc.tensor.matmul(pp[:], lhsT=S[:], rhs=rhs[:],
                             start=(k == 0), stop=(k == tile_nch - 1))
        epilogue(t, pp, sb)
        col += tile_nch


def _normalize_elu(nc, sb, pp, bias_tile, out_tile):
    """out = elu(U/den + bias) from psum [P,264] -> out_tile [P,256] SBUF."""
    den = sb.tile([P, 8], F32, tag="den")
    nc.vector.tensor_scalar_max(den[:], pp[:, FD:FD + 8], 1e-30)
    rec = sb.tile([P, 8], F32, tag="rec")
    nc.vector.reciprocal(rec[:], den[:])
    x = sb.tile([P, FD], F32, tag="xnrm")
    nc.vector.tensor_tensor(
        out=x[:].rearrange("p (h d) -> p h d", h=H),
        in0=pp[:, :FD].rearrange("p (h d) -> p h d", h=H),
        in1=rec[:, :, None].to_broadcast([P, H, 32]), op=AluOp.mult)
    nc.vector.tensor_add(x[:], x[:], bias_tile[:])
    # elu(x) = relu(x) - relu(1 - exp(x))
    ex = sb.tile([P, FD], F32, tag="eluex")
    nc.scalar.activation(ex[:], x[:], Act.Exp)
    nc.scalar.activation(ex[:], ex[:], Act.Relu, bias=1.0, scale=-1.0)
    nc.scalar.activation(out_tile[:], x[:], Act.Relu)
    nc.vector.tensor_sub(out_tile[:], out_tile[:], ex[:])


def _transpose_store(nc, sb, ps, consts, src_tile, dramT, col0):
    """src_tile [P,256] -> dramT[0:256, col0:col0+128] via two PE transposes."""
    for kk in range(2):
        tp = ps.tile([P, P], F32, space="PSUM", tag="tpps")
        nc.tensor.transpose(tp[:], src_tile[:, kk * P:(kk + 1) * P],
                            consts['ident'][:])
        ts = sb.tile([P, P], F32, tag="tpsb")
        nc.scalar.copy(ts[:], tp[:])
        nc.sync.dma_start(dramT[kk * P:(kk + 1) * P, col0:col0 + P], ts[:])


def _sem_combine(nc, sb, ps, consts, oDCT, oCCT, l, t, hc_cb):
    """Semantic attention over [o_dc, o_cc] for cell tile t (transposed).

    oDCT/oCCT: DRAM [256, CBLK]; result hcT halves passed to hc_cb(kk, tile).
    """
    W1 = consts['semW1'][l]     # 2 sbuf tiles [128,128]
    b1 = consts['semb1'][l]
    w2 = consts['semW2'][l]
    zT = []
    for src in (oDCT, oCCT):
        z0 = sb.tile([P, P], F32, tag="semz")
        z1 = sb.tile([P, P], F32, tag="semz")
        nc.sync.dma_start(z0[:], src[0:P, t * P:(t + 1) * P])
        nc.sync.dma_start(z1[:], src[P:2 * P, t * P:(t + 1) * P])
        zT.append((z0, z1))
    wms = []
    for m in range(2):
        hp = ps.tile([P, P], F32, space="PSUM", tag="aux")
        nc.tensor.matmul(hp[:], lhsT=W1[0][:], rhs=zT[m][0][:], start=True, stop=False)
        nc.tensor.matmul(hp[:], lhsT=W1[1][:], rhs=zT[m][1][:], start=False, stop=True)
        ht = sb.tile([P, P], F32, tag="semh")
        nc.scalar.activation(ht[:], hp[:], Act.Tanh, bias=b1[:])
        wp = ps.tile([1, P], F32, space="PSUM", tag="aux")
        nc.tensor.matmul(wp[:], lhsT=w2[:], rhs=ht[:], start=True, stop=True)
        wm = sb.tile([1, P], F32, tag="semw")
        nc.scalar.copy(wm[:], wp[:])
        wms.append(wm)
    beta = sb.tile([1, P], F32, tag="semb")
    nc.vector.tensor_sub(beta[:], wms[0][:], wms[1][:])
    nc.scalar.activation(beta[:], beta[:], Act.Sigmoid)
    bb = ps.tile([P, P], F32, space="PSUM", tag="aux")
    nc.tensor.matmul(bb[:], lhsT=consts['ones1'][:], rhs=beta[:], start=True, stop=True)
    for kk in range(2):
        diff = sb.tile([P, P], F32, tag="semd")
        nc.vector.tensor_sub(diff[:], zT[0][kk][:], zT[1][kk][:])
        nc.vector.tensor_mul(diff[:], diff[:], bb[:])
        hct = sb.tile([P, P], F32, tag="semhc")
        nc.vector.tensor_add(hct[:], zT[1][kk][:], diff[:])
        hc_cb(kk, hct)


def make_cfg(Nd, Nc):
    ndp = -(-Nd // 1024) * 1024
    ncp = -(-Nc // 1024) * 1024
    return dict(Nd=Nd, Nc=Nc, NDP=ndp, NCP=ncp, DBLK=ndp // 8,
                CBLK=ncp // 8, HALF=ndp // 2, n_cores=8)


def build_program(sched, cfg, legalize=True):
    nco = cfg['n_cores']
    DBLK, CBLK, HALF = cfg['DBLK'], cfg['CBLK'], cfg['HALF']
    NDP, NCP = cfg['NDP'], cfg['NCP']
    DD_T, CT, DC2_T = DBLK // P, CBLK // P, NCP // P

    nc = bacc.Bacc(None)
    d = {}
    def inp(name, shape, dt=F32):
        d[name] = nc.declare_dram_parameter(name, list(shape), dt, isOutput=False)
        return d[name]

    featDT = inp('featDT', (2, P, NDP))
    featCT = inp('featCT', (2, P, NCP))
    featDTloc = inp('featDTloc', (2, P, DBLK))
    iota_in = inp('iota', (P, P))
    ident_in = inp('ident', (P, P))
    WDD = inp('WDD', (2, P, ROWW)); WerDD = inp('WerDD', (2, P, 64))
    WDC = inp('WDC', (2, P, ROWW)); WCC1 = inp('WCC1', (2, P, ROWW))
    WDC2 = inp('WDC2', (2, P, ROWW)); WCC2 = inp('WCC2', (2, P, ROWW))
    bias5 = inp('bias5', (5, FD))
    semW1 = inp('semW1', (2, 2, P, P)); semb1 = inp('semb1', (2, P, 1))
    semW2 = inp('semW2', (2, P, 1))
    dnnW1 = inp('dnnW1', (2, P, 32)); dnnb1 = inp('dnnb1', (32, 1))
    dnnW2 = inp('dnnW2', (32, 16)); dnnb2 = inp('dnnb2', (16, 1))
    dnnW3 = inp('dnnW3', (16, 1)); dnnb3 = inp('dnnb3', (1, 1))
    streams = {}
    for r, nchs in sched.items():
        tot = int(nchs.sum())
        streams[r] = (inp(f'{r}_src', (P, tot * 8), I16),
                      inp(f'{r}_er', (P, tot * 8), I16),
                      inp(f'{r}_dl', (P, tot)))
    out = nc.declare_dram_parameter('out', [1, CBLK], F32, isOutput=True)

    with tile.TileContext(nc) as tc:
        with tc.tile_pool(name="const", bufs=1) as cpool, \
             tc.tile_pool(name="sb", bufs=3) as sb, \
             tc.tile_pool(name="gather", bufs=2) as gb, \
             tc.tile_pool(name="ps", bufs=2, space="PSUM") as ps, \
             tc.tile_pool(name="dram", bufs=1, space="DRAM") as dr:
            nc.gpsimd.load_library(library_config.mlp)

            # ---- constants in SBUF
            consts = {}
            it = cpool.tile([P, P], F32); nc.sync.dma_start(it[:], iota_in[:])
            consts['iota'] = it
            idt = cpool.tile([P, P], F32)
            nc.sync.dma_start(idt[:], ident_in[:])
            consts['ident'] = idt
            ones1 = cpool.tile([1, P], F32); nc.vector.memset(ones1[:], 1.0)
            consts['ones1'] = ones1
            def wtile(ap, ncols, nm, n=2):
                ts = []
                for k in range(n):
                    t_ = cpool.tile([P, ncols], F32, tag=f"w_{nm}_{k}")
                    nc.sync.dma_start(t_[:], ap[k, :, :ncols])
                    ts.append(t_)
                return tuple(ts)
            wt = {'WDD': wtile(WDD, ROWW, 'dd'), 'WerDD': wtile(WerDD, 64, 'erdd'),
                  'WDC': wtile(WDC, ROWW, 'dc'), 'WCC1': wtile(WCC1, ROWW, 'cc1'),
                  'WDC2': wtile(WDC2, ROWW, 'dc2'), 'WCC2': wtile(WCC2, ROWW, 'cc2')}
            btiles = []
            for r in range(5):
                bt = cpool.tile([P, FD], F32, tag=f"bias_{r}")
                nc.sync.dma_start(bt[:], bias5[r:r + 1, :].to_broadcast([P, FD]))
                btiles.append(bt)
            consts['semW1'] = []
            consts['semb1'] = []
            consts['semW2'] = []
            for l in range(2):
                t0 = cpool.tile([P, P], F32, tag=f"sw1a{l}")
                nc.sync.dma_start(t0[:], semW1[l, 0])
                t1 = cpool.tile([P, P], F32, tag=f"sw1b{l}")
                nc.sync.dma_start(t1[:], semW1[l, 1])
                consts['semW1'].append((t0, t1))
                tb = cpool.tile([P, 1], F32, tag=f"sb1{l}")
                nc.sync.dma_start(tb[:], semb1[l])
                consts['semb1'].append(tb)
                tw = cpool.tile([P, 1], F32, tag=f"sw2{l}")
                nc.sync.dma_start(tw[:], semW2[l])
                consts['semW2'].append(tw)
            dW1 = wtile(dnnW1, 32, 'dnn1')
            dW2 = cpool.tile([32, 16], F32); nc.sync.dma_start(dW2[:], dnnW2[:])
            dW3 = cpool.tile([16, 1], F32); nc.sync.dma_start(dW3[:], dnnW3[:])
            db1 = cpool.tile([32, 1], F32); nc.sync.dma_start(db1[:], dnnb1[:])
            db2 = cpool.tile([16, 1], F32); nc.sync.dma_start(db2[:], dnnb2[:])
            db3 = cpool.tile([1, 1], F32); nc.sync.dma_start(db3[:], dnnb3[:])

            # ---- internal DRAM
            tabDD = dr.tile([NDP, ROWW], F32)
            tabDC = dr.tile([NDP, ROWW], F32)
            erDD = dr.tile([DBLK, 64], F32)
            tabCC1 = dr.tile([NCP, ROWW], F32)
            tabDC2 = dr.tile([DBLK, ROWW], F32)
            tabCC2 = dr.tile([NCP, ROWW], F32)
            hd1cT = dr.tile([2 * P, DBLK], F32)
            oDC1T = dr.tile([2 * P, CBLK], F32)
            oCC1T = dr.tile([2 * P, CBLK], F32)
            oDC2T = dr.tile([2 * P, CBLK], F32)
            oCC2T = dr.tile([2 * P, CBLK], F32)
            hc1T = dr.tile([2 * P, CBLK], F32)
            hc1T_ag = dr.tile([nco * 2 * P, CBLK], F32, addr_space="Shared")
            Upart = dr.tile([NCP, 264], F32)
            Ua2a = dr.tile([NCP, 264], F32)

            # ---- L1 projections (replicated tables + local er)
            _proj_pass(nc, sb, ps, featDT, NDP // P,
                       [(tabDD.opt(), ROWW, 'WDD'), (tabDC.opt(), ROWW, 'WDC')], wt)
            _proj_pass(nc, sb, ps, featCT, NCP // P,
                       [(tabCC1.opt(), ROWW, 'WCC1')], wt)
            _proj_pass(nc, sb, ps, featDTloc, DD_T,
                       [(erDD.opt(), 64, 'WerDD')], wt)

            # ---- L1 dd edge phase -> hd1cT (transposed store)
            def dd_epi(t, pp, sb_):
                o = sb_.tile([P, FD], F32, tag="oed")
                _normalize_elu(nc, sb_, pp, btiles[0], o)
                _transpose_store(nc, sb_, ps, consts, o, hd1cT.opt(), t * P)
            _edge_phase(nc, gb, ps, consts,
                        [tabDD.opt()[0:HALF, :], tabDD.opt()[HALF:NDP, :]],
                        erDD.opt()[:, :], streams['dd'], sched['dd'], dd_epi, 0,
                        er_step=64)

            # ---- tabDC2 projection from hd1cT (local drugs)
            _proj_pass(nc, sb, ps, hd1cT.opt().rearrange("(a p) n -> a p n", p=P),
                       DD_T, [(tabDC2.opt(), ROWW, 'WDC2')], wt)

            # ---- L1 dc edge phase -> oDC1T
            def dc1_epi(t, pp, sb_):
                o = sb_.tile([P, FD], F32, tag="oed")
                _normalize_elu(nc, sb_, pp, btiles[1], o)
                _transpose_store(nc, sb_, ps, consts, o, oDC1T.opt(), t * P)
            _edge_phase(nc, gb, ps, consts,
                        [tabDC.opt()[0:HALF, :], tabDC.opt()[HALF:NDP, :]],
                        tabCC1.opt()[:, ERG:ERG + 64], streams['dc1'],
                        sched['dc1'], dc1_epi, 16)

            # ---- L1 cc edge phase -> oCC1T
            def cc1_epi(t, pp, sb_):
                o = sb_.tile([P, FD], F32, tag="oed")
                _normalize_elu(nc, sb_, pp, btiles[2], o)
                _transpose_store(nc, sb_, ps, consts, o, oCC1T.opt(), t * P)
            _edge_phase(nc, gb, ps, consts, [tabCC1.opt()[:, :]],
                        tabCC1.opt()[:, ERG:ERG + 64], streams['cc1'],
                        sched['cc1'], cc1_epi, 8)

            # ---- sem attention L1 -> hc1T, then AllGather
            for t in range(CT):
                def hc_cb(kk, hct, t=t):
                    nc.sync.dma_start(hc1T.opt()[kk * P:(kk + 1) * P,
                                                 t * P:(t + 1) * P], hct[:])
                _sem_combine(nc, sb, ps, consts, oDC1T.opt(), oCC1T.opt(),
                             0, t, hc_cb)
            nc.gpsimd.collective_compute(
                "AllGather", AluOp.bypass,
                replica_groups=[list(range(nco))],
                ins=[hc1T.opt()], outs=[hc1T_ag.opt()])

            # ---- tabCC2 projection (replicated, from AllGathered hc1T)
            agv = hc1T_ag.opt().rearrange("(c a p) n -> c a p n", a=2, p=P)
            for t in range(DC2_T):
                c_, j = t // CT, t % CT
                lh0 = sb.tile([P, P], F32, tag="projlh")
                lh1 = sb.tile([P, P], F32, tag="projlh")
                nc.sync.dma_start(lh0[:], agv[c_, 0, :, j * P:(j + 1) * P])
                nc.sync.dma_start(lh1[:], agv[c_, 1, :, j * P:(j + 1) * P])
                pp = ps.tile([P, ROWW], F32, space="PSUM", tag="projps")
                w0, w1 = wt['WCC2']
                nc.tensor.matmul(pp[:], lhsT=lh0[:], rhs=w0[:], start=True, stop=False)
                nc.tensor.matmul(pp[:], lhsT=lh1[:], rhs=w1[:], start=False, stop=True)
                ot = sb.tile([P, ROWW], F32, tag="projout")
                nc.scalar.copy(ot[:], pp[:])
                nc.sync.dma_start(tabCC2.opt()[t * P:(t + 1) * P, :], ot[:])

            # ---- L2 dc edge phase (by src) -> raw partial U
            def dc2_epi(t, pp, sb_):
                o = sb_.tile([P, 264], F32, tag="oraw")
                nc.scalar.copy(o[:], pp[:])
                nc.sync.dma_start(Upart.opt()[t * P:(t + 1) * P, :], o[:])
            _edge_phase(nc, gb, ps, consts, [tabDC2.opt()[:, :]],
                        tabCC2.opt()[:, ERG:ERG + 64], streams['dc2'],
                        sched['dc2'], dc2_epi, 16)

            # ---- AllToAll partial U; local reduce + normalize -> oDC2T
            nc.gpsimd.collective_compute(
                "AllToAll", AluOp.bypass,
                replica_groups=[list(range(nco))],
                ins=[Upart.opt()], outs=[Ua2a.opt()])
            a2av = Ua2a.opt().rearrange("(c r) n -> c r n", c=nco)
            for t in range(CT):
                acc = sb.tile([P, 264], F32, tag="acc")
                tmp = sb.tile([P, 264], F32, tag="acct")
                nc.sync.dma_start(acc[:], a2av[0, t * P:(t + 1) * P, :])
                for j in range(1, nco):
                    nc.sync.dma_start(tmp[:], a2av[j, t * P:(t + 1) * P, :])
                    nc.vector.tensor_add(acc[:], acc[:], tmp[:])
                    tmp = sb.tile([P, 264], F32, tag="acct")
                o = sb.tile([P, FD], F32, tag="oed")
                _normalize_elu_sbuf(nc, sb, acc, btiles[3], o)
                _transpose_store(nc, sb, ps, consts, o, oDC2T.opt(), t * P)

            # ---- L2 cc edge phase -> oCC2T
            def cc2_epi(t, pp, sb_):
                o = sb_.tile([P, FD], F32, tag="oed")
                _normalize_elu(nc, sb_, pp, btiles[4], o)
                _transpose_store(nc, sb_, ps, consts, o, oCC2T.opt(), t * P)
            _edge_phase(nc, gb, ps, consts, [tabCC2.opt()[:, :]],
                        tabCC2.opt()[:, ERG:ERG + 64], streams['cc2'],
                        sched['cc2'], cc2_epi, 8)

            # ---- sem attention L2 + MLP head
            for t in range(CT):
                hct_tiles = {}
                def hc2_cb(kk, hct, bag=hct_tiles):
                    bag[kk] = hct
                _sem_combine(nc, sb, ps, consts, oDC2T.opt(), oCC2T.opt(),
                             1, t, hc2_cb)
                h1p = ps.tile([32, P], F32, space="PSUM", tag="aux")
                nc.tensor.matmul(h1p[:], lhsT=dW1[0][:], rhs=hct_tiles[0][:],
                                 start=True, stop=False)
                nc.tensor.matmul(h1p[:], lhsT=dW1[1][:], rhs=hct_tiles[1][:],
                                 start=False, stop=True)
                h1 = sb.tile([32, P], F32, tag="mlph1")
                nc.vector.scalar_tensor_tensor(
                    out=h1[:], in0=h1p[:], scalar=1.0, in1=db1[:].to_broadcast([32, P]),
                    op0=AluOp.mult, op1=AluOp.add)
                nc.vector.scalar_tensor_tensor(
                    out=h1[:], in0=h1[:], scalar=0.01, in1=h1[:],
                    op0=AluOp.mult, op1=AluOp.max)
                h2p = ps.tile([16, P], F32, space="PSUM", tag="aux")
                nc.tensor.matmul(h2p[:], lhsT=dW2[:], rhs=h1[:], start=True, stop=True)
                h2 = sb.tile([16, P], F32, tag="mlph2")
                nc.vector.scalar_tensor_tensor(
                    out=h2[:], in0=h2p[:], scalar=1.0, in1=db2[:].to_broadcast([16, P]),
                    op0=AluOp.mult, op1=AluOp.add)
                nc.vector.scalar_tensor_tensor(
                    out=h2[:], in0=h2[:], scalar=0.01, in1=h2[:],
                    op0=AluOp.mult, op1=AluOp.max)
                h3p = ps.tile([1, P], F32, space="PSUM", tag="aux")
                nc.tensor.matmul(h3p[:], lhsT=dW3[:], rhs=h2[:], start=True, stop=True)
                h3 = sb.tile([1, P], F32, tag="mlph3")
                nc.vector.tensor_scalar(h3[:], h3p[:], db3[:], None, op0=AluOp.add)
                nc.sync.dma_start(out[0:1, t * P:(t + 1) * P], h3[:])

    nc.compile()
    if legalize:
        legalize_waits(nc)
    return nc


def _normalize_elu_sbuf(nc, sb, acc, bias_tile, out_tile):
    """Same as _normalize_elu but source is an SBUF tile [P,264]."""
    den = sb.tile([P, 8], F32, tag="den")
    nc.vector.tensor_scalar_max(den[:], acc[:, FD:FD + 8], 1e-30)
    rec = sb.tile([P, 8], F32, tag="rec")
    nc.vector.reciprocal(rec[:], den[:])
    x = sb.tile([P, FD], F32, tag="xnrm")
    nc.vector.tensor_tensor(
        out=x[:].rearrange("p (h d) -> p h d", h=H),
        in0=acc[:, :FD].rearrange("p (h d) -> p h d", h=H),
        in1=rec[:, :, None].to_broadcast([P, H, 32]), op=AluOp.mult)
    nc.vector.tensor_add(x[:], x[:], bias_tile[:])
    ex = sb.tile([P, FD], F32, tag="eluex")
    nc.scalar.activation(ex[:], x[:], Act.Exp)
    nc.scalar.activation(ex[:], ex[:], Act.Relu, bias=1.0, scale=-1.0)
    nc.scalar.activation(out_tile[:], x[:], Act.Relu)
    nc.vector.tensor_sub(out_tile[:], out_tile[:], ex[:])


# --------------------------------------------------------------------------
# entry point
# --------------------------------------------------------------------------

_CACHE = {}


def kernel(**inputs):
    cfg = make_cfg(inputs['feat_drug'].shape[0], inputs['feat_cell'].shape[0])
    sched, in_maps = host_prep(inputs, cfg)
    key = tuple(int(x) for s in sched.values() for x in s.flatten())
    if key not in _CACHE:
        _CACHE[key] = build_program(sched, cfg)
    nc = _CACHE[key]
    from concourse.bass_utils import run_bass_kernel_spmd
    res = run_bass_kernel_spmd(nc, in_maps, list(range(cfg['n_cores'])))
    pieces = [res.results[c]['out'][0] for c in range(cfg['n_cores'])]
    full = np.concatenate([p[:cfg['CBLK']] for p in pieces])[:cfg['Nc']]
    return full.reshape(-1, 1).astype(np.float32)

